# revision 18
# baseline (speedup 1.0000x reference)
"""Causal MHA (RoPE, 16 heads, D=1024, S=2048, B=2) on 8 trn2 NeuronCores.

Sharding: data-parallel over batch (2 groups of 4 cores) x tensor-parallel
over heads (4 heads / core). Each core computes q/k/v projections for its
256 output dims, RoPE, causal attention for its 4 heads, and a partial
output projection y_c = out_c @ Wo[:, slice].T. Host sums the 4 partials
per batch (row-parallel unshard).

All matmuls run in float32r (TF32-like, ~1e-4 rel err, full PE rate at
free-dim >= 256). Scores are computed transposed ([keys, q]) so the
attention @V matmul has q as its 512-wide free dim, and softmax
denominators come free as an extra ones-column in the V operand.
"""

import numpy as np

D_MODEL = 1024
S = 2048
NH = 16
HD = 64
THETA = 10000.0
HPC = 4          # heads per core
DPC = HPC * HD   # dims per core = 256
NG = 2           # dim groups of 128 (pairs of heads)
W = 512          # q-block width
NKO = D_MODEL // 128
NTC = S // 128   # 16 token chunks of 128
HALF = S // 2
MASK_VAL = -1e9

_CACHE = {}


def _build_nc():
    import concourse.bass as bass
    import concourse.tile as tile
    from concourse import bacc, mybir
    from contextlib import ExitStack

    F32 = mybir.dt.float32
    F32R = mybir.dt.float32r
    AF = mybir.ActivationFunctionType
    ts = bass.ts

    nc = bacc.Bacc(None, target_bir_lowering=False)
    xT = nc.dram_tensor("xT", [D_MODEL, S], F32, kind="ExternalInput")
    wq = nc.dram_tensor("wq", [D_MODEL, DPC], F32, kind="ExternalInput")
    wk = nc.dram_tensor("wk", [D_MODEL, DPC], F32, kind="ExternalInput")
    wv = nc.dram_tensor("wv", [D_MODEL, DPC], F32, kind="ExternalInput")
    wo = nc.dram_tensor("wo", [DPC, D_MODEL], F32, kind="ExternalInput")
    coss = nc.dram_tensor("coss", [128, S], F32, kind="ExternalInput")
    sins = nc.dram_tensor("sins", [128, S], F32, kind="ExternalInput")
    pmat = nc.dram_tensor("pmat", [128, 128], F32, kind="ExternalInput")
    mask = nc.dram_tensor("mask", [128, 128], F32, kind="ExternalInput")
    ident = nc.dram_tensor("ident", [128, 128], F32, kind="ExternalInput")
    y = nc.dram_tensor("y", [S, D_MODEL], F32, kind="ExternalOutput")

    with tile.TileContext(nc) as tc, ExitStack() as ctx:
        const = ctx.enter_context(tc.tile_pool(name="const", bufs=1))
        persist = ctx.enter_context(tc.tile_pool(name="persist", bufs=1))

        # ---- constants (DMAs deferred into phase 1 to not hog queues) --
        BF16 = mybir.dt.bfloat16
        ones_raw = const.tile([128, HPC], F32)
        nc.gpsimd.memset(ones_raw[:], 1.0)
        ones_r = const.tile([128, HPC], F32R)
        nc.vector.tensor_copy(ones_r[:], ones_raw[:])

        # persistent activations
        qT = [persist.tile([128, S], F32R, name=f"qT{g}") for g in range(NG)]
        kT = [persist.tile([128, S], F32R, name=f"kT{g}") for g in range(NG)]
        v_aug = persist.tile([128, NTC, HPC * (HD + 1)], F32R, name="v_aug")
        out_cT = [persist.tile([128, S], F32R, name=f"out_cT{g}")
                  for g in range(NG)]
        wo_r = persist.tile([128, NG, D_MODEL], F32R, name="wo_r")

        # ---- fused pipeline: per quarter hf: QKV(hf) then ATTN(qb=hf) --
        with nc.named_scope("fused"), \
             tc.tile_pool(name="qkvw", bufs=1) as wpool, \
             tc.tile_pool(name="qkv", bufs=2) as qkv_pool, \
             tc.tile_pool(name="xtr", bufs=2) as xt_pool, \
             tc.tile_pool(name="att", bufs=2) as att_pool, \
             tc.tile_pool(name="norm", bufs=2) as norm_pool:

            def load_round(name, dram, width):
                t = wpool.tile([128, NKO, width], F32R, name=name + "_r")
                for ko in range(NKO):
                    stg = qkv_pool.tile([128, width], F32, tag="x_stg",
                                        name="w_stg")
                    nc.sync.dma_start(stg[:], dram.ap()[ts(ko, 128), :])
                    nc.vector.tensor_copy(t[:, ko], stg[:])
                return t

            def load_x_quarter(hf):
                xr = xt_pool.tile([128, NKO, W], F32R, tag="xT_r",
                                  name="xT_r")
                for ko in range(NKO):
                    stg = qkv_pool.tile([128, W], F32, tag="x_stg",
                                        name="x_stg")
                    nc.sync.dma_start(stg[:], xT.ap()[ts(ko, 128), ts(hf, W)])
                    nc.vector.tensor_copy(xr[:, ko], stg[:])
                return xr

            wv_r = load_round("wv", wv, DPC)
            xquart = load_x_quarter(0)
            wq_r = load_round("wq", wq, DPC)
            wk_r = load_round("wk", wk, DPC)

            pm_raw = const.tile([128, 128], F32)
            nc.sync.dma_start(pm_raw[:], pmat.ap())
            pm_r = const.tile([128, 128], F32R)
            nc.scalar.copy(pm_r[:], pm_raw[:])
            msk_raw = const.tile([128, 128], F32)
            nc.sync.dma_start(msk_raw[:], mask.ap())
            msk_r = const.tile([128, 128], BF16)
            nc.scalar.copy(msk_r[:], msk_raw[:])
            id_raw = const.tile([128, 128], F32)
            nc.sync.dma_start(id_raw[:], ident.ap())
            id_r = const.tile([128, 128], BF16)
            nc.scalar.copy(id_r[:], id_raw[:])
            cs_t = const.tile([128, S], F32)
            sn_t = const.tile([128, S], F32)
            for j4 in range(4):
                nc.sync.dma_start(cs_t[:, ts(j4, W)], coss.ap()[:, ts(j4, W)])
                nc.sync.dma_start(sn_t[:, ts(j4, W)], sins.ap()[:, ts(j4, W)])
            for g2 in range(NG):
                wos = const.tile([128, D_MODEL], F32, tag="wo_stg", name="wos")
                nc.sync.dma_start(wos[:], wo.ap()[ts(g2, 128), :])
                nc.vector.tensor_copy(wo_r[:, g2], wos[:])

            pending = []

            def emit_oproj(tcN, pspool):
                for e2 in range(2):
                    psy = pspool.tile([128, W], F32, tag="sc", name="psy")
                    for g in range(NG):
                        nc.tensor.matmul(psy[:], out_cT[g][:, ts(tcN, 128)],
                                         wo_r[:, g, ts(e2, W)],
                                         start=(g == 0), stop=(g == NG - 1),
                                         skip_group_check=True)
                    ysb = norm_pool.tile([128, W], F32, tag="ysb", name="ysb")
                    nc.vector.tensor_copy(ysb[:], psy[:])
                    nc.sync.dma_start(y.ap()[ts(tcN, 128), ts(e2, W)],
                                      ysb[:])

            for hf in range(4):
                # ---- QKV for quarter hf ------------------------------
                with tc.tile_pool(name="ps1v", bufs=2, space="PSUM") as ps1v, \
                     tc.tile_pool(name="ps1qk", bufs=4, space="PSUM") as ps1qk, \
                     tc.tile_pool(name="ps1p", bufs=2, space="PSUM") as ps1p:
                    xT_r = xquart
                    for tl in range(W // 128):
                        tcN = hf * (W // 128) + tl
                        psv = ps1v.tile([128, DPC], F32, tag="psv", name="psv")
                        for ko in range(NKO):
                            nc.tensor.matmul(psv[:], xT_r[:, ko, ts(tl, 128)],
                                             wv_r[:, ko],
                                             start=(ko == 0),
                                             stop=(ko == NKO - 1))
                        for h in range(HPC):
                            nc.scalar.copy(
                                v_aug[:, tcN, h * (HD + 1):h * (HD + 1) + HD],
                                psv[:, ts(h, HD)])
                        nc.vector.tensor_copy(v_aug[:, tcN, HD::HD + 1],
                                              ones_r[:])

                    for g in range(NG):
                        t4 = hf
                        psq = ps1qk.tile([128, W], F32, tag="psqk", name="psq")
                        for ko in range(NKO):
                            nc.tensor.matmul(
                                psq[:], wq_r[:, ko, ts(g, 128)], xT_r[:, ko],
                                start=(ko == 0), stop=(ko == NKO - 1))
                        rawq = qkv_pool.tile([128, W], F32R, tag="rawq",
                                             name="rawq")
                        nc.scalar.copy(rawq[:], psq[:])
                        psk = ps1qk.tile([128, W], F32, tag="psqk", name="psk")
                        for ko in range(NKO):
                            nc.tensor.matmul(
                                psk[:], wk_r[:, ko, ts(g, 128)], xT_r[:, ko],
                                start=(ko == 0), stop=(ko == NKO - 1))
                        rawk = qkv_pool.tile([128, W], F32R, tag="rawk",
                                             name="rawk")
                        nc.scalar.copy(rawk[:], psk[:])
                        for nm, raw, dst in (("q", rawq, qT[g]),
                                             ("k", rawk, kT[g])):
                            psp = ps1p.tile([128, W], F32, tag="psp",
                                            name="psp")
                            nc.tensor.matmul(psp[:], pm_r[:], raw[:],
                                             start=True, stop=True)
                            t1 = qkv_pool.tile([128, W], F32, tag="t1",
                                               name="t1")
                            nc.gpsimd.tensor_tensor(t1[:], raw[:],
                                                    cs_t[:, ts(t4, W)],
                                                    mybir.AluOpType.mult)
                            t2 = qkv_pool.tile([128, W], F32, tag="t2",
                                               name="t2")
                            nc.vector.tensor_tensor(t2[:], psp[:],
                                                    sn_t[:, ts(t4, W)],
                                                    mybir.AluOpType.mult)
                            nc.vector.tensor_tensor(dst[:, ts(t4, W)],
                                                    t1[:], t2[:],
                                                    mybir.AluOpType.add)

                # ---- attention for qb = hf (+ x prefetch for hf+1) ----
                qb = hf
                with tc.tile_pool(name="ps2", bufs=2, space="PSUM") as ps2, \
                     tc.tile_pool(name="ps2av", bufs=1, space="PSUM") as ps2av:
                    if hf < 3:
                        xquart = load_x_quarter(hf + 1)
                    av = [ps2av.tile([HD + 1, W], F32, tag=f"av{hh}",
                                     name=f"av{hh}") for hh in range(4)]
                    nkb = (qb + 1) * (W // 128)
                    for kb in range(nkb):
                        if kb >= 2 and pending:
                            emit_oproj(pending.pop(0), ps2)
                        cs0 = max(0, kb * 128 - qb * W)
                        diag = kb * 128 >= qb * W
                        for g in range(NG):
                            sc = ps2.tile([128, 2 * W], F32, tag="sc",
                                          name="sc")
                            for h in range(2):
                                nc.tensor.matmul(
                                    sc[:, h * W + cs0:(h + 1) * W],
                                    kT[g][ts(h, HD), ts(kb, 128)],
                                    qT[g][ts(h, HD),
                                          qb * W + cs0:(qb + 1) * W],
                                    start=True, stop=not diag,
                                    skip_group_check=True)
                                if diag:
                                    nc.tensor.matmul(
                                        sc[:, h * W + cs0:h * W + cs0 + 128],
                                        id_r[:], msk_r[:],
                                        start=False, stop=True,
                                        skip_group_check=True)
                            att = att_pool.tile([128, 2 * W], F32R,
                                                tag="attw", name="att")
                            scv = sc[:].rearrange("p (h w) -> p h w", h=2)
                            atv = att[:].rearrange("p (h w) -> p h w", h=2)
                            nc.scalar.activation(atv[:, :, cs0:],
                                                 scv[:, :, cs0:],
                                                 AF.Exp,
                                                 scale=1.0 / np.sqrt(HD))
                            for h in range(2):
                                hh = 2 * g + h
                                nc.tensor.matmul(
                                    av[hh][:, cs0:],
                                    v_aug[:, kb, hh * (HD + 1):
                                          (hh + 1) * (HD + 1)],
                                    att[:, h * W + cs0:(h + 1) * W],
                                    start=(kb == 0), stop=(kb == nkb - 1),
                                    skip_group_check=True)
                    rss = []
                    for hh in range(4):
                        rs = norm_pool.tile([1, W], F32, tag=f"rs{hh}",
                                            name="rs")
                        nc.vector.tensor_copy(rs[:], av[hh][HD:HD + 1, :])
                        rss.append(rs)
                    for hh in range(4):
                        g, h = divmod(hh, 2)
                        rec = norm_pool.tile([1, W], F32, tag="rec",
                                             name="rec")
                        nc.vector.reciprocal_approx_fast(rec[:], rss[hh][:])
                        rb = norm_pool.tile([HD, W], F32, tag="rb", name="rb")
                        nc.gpsimd.partition_broadcast(rb[:], rec[:])
                        nc.vector.tensor_tensor(
                            out_cT[g][ts(h, HD), ts(qb, W)],
                            av[hh][0:HD, :], rb[:], mybir.AluOpType.mult)
                    pending.extend(qb * (W // 128) + tl
                                   for tl in range(W // 128))
                    if hf == 3:
                        for tcN in pending:
                            emit_oproj(tcN, ps2)
                        pending = []

    nc.compile()
    return nc


def _host_inputs():
    d = HD
    inv_freq = THETA ** (-np.arange(0, d, 2, dtype=np.float64) / d)  # [32]
    t = np.arange(S, dtype=np.float64)
    ang = t[None, :] * inv_freq[:, None]          # [32, S]
    C64 = np.repeat(np.cos(ang), 2, axis=0)       # [64, S] per-dim cos
    S64 = np.repeat(np.sin(ang), 2, axis=0).copy()
    S64[0::2] *= -1.0                             # even dims: -sin
    C = np.tile(C64, (2, 1)).astype(np.float32)   # [128, S] two heads
    Sg = np.tile(S64, (2, 1)).astype(np.float32)

    P = np.zeros((128, 128), np.float32)
    idx = np.arange(128)
    P[idx ^ 1, idx] = 1.0

    M = np.where(np.arange(128)[None, :] >= np.arange(128)[:, None],
                 0.0, MASK_VAL).astype(np.float32)
    ident = np.eye(128, dtype=np.float32)
    return C, Sg, P, M, ident


def kernel(x, Wq, Wk, Wv, Wo):
    from concourse.bass_utils import run_bass_kernel_spmd

    x = np.asarray(x, np.float32)
    Wq = np.asarray(Wq, np.float32)
    Wk = np.asarray(Wk, np.float32)
    Wv = np.asarray(Wv, np.float32)
    Wo = np.asarray(Wo, np.float32)
    B = x.shape[0]

    if "nc" not in _CACHE:
        _CACHE["nc"] = _build_nc()
    nc = _CACHE["nc"]

    C, Sg, P, M, ident = _host_inputs()
    xTb = [np.ascontiguousarray(x[b].T) for b in range(B)]
    in_maps = []
    for c in range(8):
        b, hq = divmod(c, 4)
        sl = slice(hq * DPC, (hq + 1) * DPC)
        in_maps.append({
            "xT": xTb[b],
            "wq": np.ascontiguousarray(Wq[sl, :].T),
            "wk": np.ascontiguousarray(Wk[sl, :].T),
            "wv": np.ascontiguousarray(Wv[sl, :].T),
            "wo": np.ascontiguousarray(Wo[:, sl].T),
            "coss": C, "sins": Sg, "pmat": P, "mask": M, "ident": ident,
        })

    res = run_bass_kernel_spmd(nc, in_maps, list(range(8)), **_CACHE.get("runkw", {}))
    _CACHE["last_res"] = res
    out = np.zeros((B, S, D_MODEL), np.float32)
    for c in range(8):
        b = c // 4
        out[b] += res.results[c]["y"]
    return out


# revision 19
# speedup vs baseline: 1.0429x; 1.0429x over previous
"""Causal MHA (RoPE, 16 heads, D=1024, S=2048, B=2) on 8 trn2 NeuronCores.

Sharding: data-parallel over batch (2 groups of 4 cores) x tensor-parallel
over heads (4 heads / core). Each core computes q/k/v projections for its
256 output dims, RoPE, causal attention for its 4 heads, and a partial
output projection y_c = out_c @ Wo[:, slice].T. Host sums the 4 partials
per batch (row-parallel unshard).

All matmuls run in float32r (TF32-like, ~1e-4 rel err, full PE rate at
free-dim >= 256). Scores are computed transposed ([keys, q]) so the
attention @V matmul has q as its 512-wide free dim, and softmax
denominators come free as an extra ones-column in the V operand.
"""

import numpy as np

D_MODEL = 1024
S = 2048
NH = 16
HD = 64
THETA = 10000.0
HPC = 4          # heads per core
DPC = HPC * HD   # dims per core = 256
NG = 2           # dim groups of 128 (pairs of heads)
W = 512          # q-block width
NKO = D_MODEL // 128
NTC = S // 128   # 16 token chunks of 128
HALF = S // 2
MASK_VAL = -1e9

_CACHE = {}


def _build_nc():
    import concourse.bass as bass
    import concourse.tile as tile
    from concourse import bacc, mybir
    from contextlib import ExitStack

    F32 = mybir.dt.float32
    F32R = mybir.dt.float32r
    AF = mybir.ActivationFunctionType
    ts = bass.ts

    nc = bacc.Bacc(None, target_bir_lowering=False)
    xT = nc.dram_tensor("xT", [D_MODEL, S], F32, kind="ExternalInput")
    wq = nc.dram_tensor("wq", [D_MODEL, DPC], F32, kind="ExternalInput")
    wk = nc.dram_tensor("wk", [D_MODEL, DPC], F32, kind="ExternalInput")
    wv = nc.dram_tensor("wv", [D_MODEL, DPC], F32, kind="ExternalInput")
    wo = nc.dram_tensor("wo", [DPC, D_MODEL], F32, kind="ExternalInput")
    coss = nc.dram_tensor("coss", [128, S], F32, kind="ExternalInput")
    sins = nc.dram_tensor("sins", [128, S], F32, kind="ExternalInput")
    pmat = nc.dram_tensor("pmat", [128, 128], F32, kind="ExternalInput")
    mask = nc.dram_tensor("mask", [128, 128], F32, kind="ExternalInput")
    ident = nc.dram_tensor("ident", [128, 128], F32, kind="ExternalInput")
    y = nc.dram_tensor("y", [S, D_MODEL], F32, kind="ExternalOutput")

    with tile.TileContext(nc) as tc, ExitStack() as ctx:
        const = ctx.enter_context(tc.tile_pool(name="const", bufs=1))
        persist = ctx.enter_context(tc.tile_pool(name="persist", bufs=1))

        # ---- constants (DMAs deferred into phase 1 to not hog queues) --
        BF16 = mybir.dt.bfloat16
        ones_raw = const.tile([128, HPC], F32)
        nc.gpsimd.memset(ones_raw[:], 1.0)
        ones_r = const.tile([128, HPC], F32R)
        nc.vector.tensor_copy(ones_r[:], ones_raw[:])

        # persistent activations
        qT = [persist.tile([128, S], F32R, name=f"qT{g}") for g in range(NG)]
        kT = [persist.tile([128, S], F32R, name=f"kT{g}") for g in range(NG)]
        v_aug = persist.tile([128, NTC, HPC * (HD + 1)], F32R, name="v_aug")
        out_cT = [persist.tile([128, S], F32R, name=f"out_cT{g}")
                  for g in range(NG)]
        wo_r = persist.tile([128, NG, D_MODEL], F32R, name="wo_r")

        # ---- phase 1: QKV + RoPE (x streamed in 4 quarters) -----------
        with nc.named_scope("qkv"), \
             tc.tile_pool(name="qkvw", bufs=1) as wpool, \
             tc.tile_pool(name="qkv", bufs=2) as qkv_pool, \
             tc.tile_pool(name="xtr", bufs=2) as xt_pool, \
             tc.tile_pool(name="ps1v", bufs=2, space="PSUM") as ps1v, \
             tc.tile_pool(name="ps1qk", bufs=4, space="PSUM") as ps1qk, \
             tc.tile_pool(name="ps1p", bufs=2, space="PSUM") as ps1p:

            def load_round(name, dram, width):
                t = wpool.tile([128, NKO, width], F32R, name=name + "_r")
                for ko in range(NKO):
                    stg = qkv_pool.tile([128, width], F32, tag="x_stg",
                                        name="w_stg")
                    nc.sync.dma_start(stg[:], dram.ap()[ts(ko, 128), :])
                    nc.vector.tensor_copy(t[:, ko], stg[:])
                return t

            def load_x_quarter(hf):
                xr = xt_pool.tile([128, NKO, W], F32R, tag="xT_r",
                                  name="xT_r")
                for ko in range(NKO):
                    stg = qkv_pool.tile([128, W], F32, tag="x_stg",
                                        name="x_stg")
                    nc.sync.dma_start(stg[:], xT.ap()[ts(ko, 128), ts(hf, W)])
                    nc.vector.tensor_copy(xr[:, ko], stg[:])
                return xr

            wv_r = load_round("wv", wv, DPC)
            xquart = load_x_quarter(0)
            wq_r = load_round("wq", wq, DPC)
            wk_r = load_round("wk", wk, DPC)

            pm_raw = const.tile([128, 128], F32)
            nc.sync.dma_start(pm_raw[:], pmat.ap())
            pm_r = const.tile([128, 128], F32R)
            nc.scalar.copy(pm_r[:], pm_raw[:])
            msk_raw = const.tile([128, 128], F32)
            nc.sync.dma_start(msk_raw[:], mask.ap())
            msk_r = const.tile([128, 128], BF16)
            nc.scalar.copy(msk_r[:], msk_raw[:])
            id_raw = const.tile([128, 128], F32)
            nc.sync.dma_start(id_raw[:], ident.ap())
            id_r = const.tile([128, 128], BF16)
            nc.scalar.copy(id_r[:], id_raw[:])
            cs_t = const.tile([128, S], F32)
            sn_t = const.tile([128, S], F32)
            for j4 in range(4):
                nc.sync.dma_start(cs_t[:, ts(j4, W)], coss.ap()[:, ts(j4, W)])
                nc.sync.dma_start(sn_t[:, ts(j4, W)], sins.ap()[:, ts(j4, W)])
            for g2 in range(NG):
                wos = const.tile([128, D_MODEL], F32, tag="wo_stg", name="wos")
                nc.sync.dma_start(wos[:], wo.ap()[ts(g2, 128), :])
                nc.vector.tensor_copy(wo_r[:, g2], wos[:])

            for hf in range(4):
                xT_r = xquart
                if hf < 3:
                    xquart = load_x_quarter(hf + 1)

                for tl in range(W // 128):
                    tcN = hf * (W // 128) + tl
                    psv = ps1v.tile([128, DPC], F32, tag="psv", name="psv")
                    for ko in range(NKO):
                        nc.tensor.matmul(psv[:], xT_r[:, ko, ts(tl, 128)],
                                         wv_r[:, ko],
                                         start=(ko == 0), stop=(ko == NKO - 1))
                    for h in range(HPC):
                        nc.scalar.copy(
                            v_aug[:, tcN, h * (HD + 1):h * (HD + 1) + HD],
                            psv[:, ts(h, HD)])
                    nc.vector.tensor_copy(v_aug[:, tcN, HD::HD + 1], ones_r[:])

                for g in range(NG):
                    t4 = hf
                    psq = ps1qk.tile([128, W], F32, tag="psqk", name="psq")
                    for ko in range(NKO):
                        nc.tensor.matmul(
                            psq[:], wq_r[:, ko, ts(g, 128)], xT_r[:, ko],
                            start=(ko == 0), stop=(ko == NKO - 1))
                    rawq = qkv_pool.tile([128, W], F32R, tag="rawq",
                                         name="rawq")
                    nc.scalar.copy(rawq[:], psq[:])
                    psk = ps1qk.tile([128, W], F32, tag="psqk", name="psk")
                    for ko in range(NKO):
                        nc.tensor.matmul(
                            psk[:], wk_r[:, ko, ts(g, 128)], xT_r[:, ko],
                            start=(ko == 0), stop=(ko == NKO - 1))
                    rawk = qkv_pool.tile([128, W], F32R, tag="rawk",
                                         name="rawk")
                    nc.scalar.copy(rawk[:], psk[:])
                    for nm, raw, dst in (("q", rawq, qT[g]),
                                         ("k", rawk, kT[g])):
                        psp = ps1p.tile([128, W], F32, tag="psp", name="psp")
                        nc.tensor.matmul(psp[:], pm_r[:], raw[:],
                                         start=True, stop=True)
                        t1 = qkv_pool.tile([128, W], F32, tag="t1",
                                           name="t1")
                        nc.gpsimd.tensor_tensor(t1[:], raw[:],
                                                cs_t[:, ts(t4, W)],
                                                mybir.AluOpType.mult)
                        t2 = qkv_pool.tile([128, W], F32, tag="t2",
                                           name="t2")
                        nc.vector.tensor_tensor(t2[:], psp[:],
                                                sn_t[:, ts(t4, W)],
                                                mybir.AluOpType.mult)
                        nc.vector.tensor_tensor(dst[:, ts(t4, W)],
                                                t1[:], t2[:],
                                                mybir.AluOpType.add)

        # ---- phase 2: attention (+ interleaved output projection) -----
        with nc.named_scope("attn"), \
             tc.tile_pool(name="att", bufs=6) as att_pool, \
             tc.tile_pool(name="norm", bufs=3) as norm_pool, \
             tc.tile_pool(name="ps2", bufs=2, space="PSUM") as ps2, \
             tc.tile_pool(name="ps2av", bufs=1, space="PSUM") as ps2av:
            pending = []

            def emit_oproj(tcN):
                ysb = norm_pool.tile([128, D_MODEL], F32, tag="ysb",
                                     name="ysb")
                for e2 in range(2):
                    psy = ps2.tile([128, W], F32, tag="sc", name="psy")
                    for g in range(NG):
                        nc.tensor.matmul(psy[:], out_cT[g][:, ts(tcN, 128)],
                                         wo_r[:, g, ts(e2, W)],
                                         start=(g == 0), stop=(g == NG - 1),
                                         skip_group_check=True)
                    nc.vector.tensor_copy(ysb[:, ts(e2, W)], psy[:])
                nc.sync.dma_start(y.ap()[ts(tcN, 128), :], ysb[:])

            for qb in range(S // W):
                av = [ps2av.tile([HD + 1, W], F32, tag=f"av{hh}",
                                 name=f"av{hh}") for hh in range(4)]
                nkb = (qb + 1) * (W // 128)
                for kb in range(nkb):
                    if kb >= 2 and pending:
                        emit_oproj(pending.pop(0))
                    cs0 = max(0, kb * 128 - qb * W)
                    diag = kb * 128 >= qb * W
                    for g in range(NG):
                        sc = ps2.tile([128, 2 * W], F32, tag="sc", name="sc")
                        for h in range(2):
                            nc.tensor.matmul(
                                sc[:, h * W + cs0:(h + 1) * W],
                                kT[g][ts(h, HD), ts(kb, 128)],
                                qT[g][ts(h, HD), qb * W + cs0:(qb + 1) * W],
                                start=True, stop=not diag,
                                skip_group_check=True)
                            if diag:
                                nc.tensor.matmul(
                                    sc[:, h * W + cs0:h * W + cs0 + 128],
                                    id_r[:], msk_r[:],
                                    start=False, stop=True,
                                    skip_group_check=True)
                        att = att_pool.tile([128, 2 * W], F32R, tag="attw",
                                            name="att")
                        scv = sc[:].rearrange("p (h w) -> p h w", h=2)
                        atv = att[:].rearrange("p (h w) -> p h w", h=2)
                        nc.scalar.activation(atv[:, :, cs0:], scv[:, :, cs0:],
                                             AF.Exp, scale=1.0 / np.sqrt(HD))
                        for h in range(2):
                            hh = 2 * g + h
                            nc.tensor.matmul(
                                av[hh][:, cs0:],
                                v_aug[:, kb, hh * (HD + 1):
                                      (hh + 1) * (HD + 1)],
                                att[:, h * W + cs0:(h + 1) * W],
                                start=(kb == 0), stop=(kb == nkb - 1),
                                skip_group_check=True)
                rss = []
                for hh in range(4):
                    rs = norm_pool.tile([1, W], F32, tag=f"rs{hh}", name="rs")
                    nc.vector.tensor_copy(rs[:], av[hh][HD:HD + 1, :])
                    rss.append(rs)
                for hh in range(4):
                    g, h = divmod(hh, 2)
                    rec = norm_pool.tile([1, W], F32, tag="rec", name="rec")
                    nc.vector.reciprocal_approx_fast(rec[:], rss[hh][:])
                    rb = norm_pool.tile([HD, W], F32, tag="rb", name="rb")
                    nc.gpsimd.partition_broadcast(rb[:], rec[:])
                    nc.vector.tensor_tensor(
                        out_cT[g][ts(h, HD), ts(qb, W)],
                        av[hh][0:HD, :], rb[:], mybir.AluOpType.mult)
                pending.extend(qb * (W // 128) + tl for tl in range(W // 128))
            for tcN in pending:
                emit_oproj(tcN)

    nc.compile()
    return nc


def _host_inputs():
    d = HD
    inv_freq = THETA ** (-np.arange(0, d, 2, dtype=np.float64) / d)  # [32]
    t = np.arange(S, dtype=np.float64)
    ang = t[None, :] * inv_freq[:, None]          # [32, S]
    C64 = np.repeat(np.cos(ang), 2, axis=0)       # [64, S] per-dim cos
    S64 = np.repeat(np.sin(ang), 2, axis=0).copy()
    S64[0::2] *= -1.0                             # even dims: -sin
    C = np.tile(C64, (2, 1)).astype(np.float32)   # [128, S] two heads
    Sg = np.tile(S64, (2, 1)).astype(np.float32)

    P = np.zeros((128, 128), np.float32)
    idx = np.arange(128)
    P[idx ^ 1, idx] = 1.0

    M = np.where(np.arange(128)[None, :] >= np.arange(128)[:, None],
                 0.0, MASK_VAL).astype(np.float32)
    ident = np.eye(128, dtype=np.float32)
    return C, Sg, P, M, ident


def kernel(x, Wq, Wk, Wv, Wo):
    from concourse.bass_utils import run_bass_kernel_spmd

    x = np.asarray(x, np.float32)
    Wq = np.asarray(Wq, np.float32)
    Wk = np.asarray(Wk, np.float32)
    Wv = np.asarray(Wv, np.float32)
    Wo = np.asarray(Wo, np.float32)
    B = x.shape[0]

    if "nc" not in _CACHE:
        _CACHE["nc"] = _build_nc()
    nc = _CACHE["nc"]

    C, Sg, P, M, ident = _host_inputs()
    xTb = [np.ascontiguousarray(x[b].T) for b in range(B)]
    in_maps = []
    for c in range(8):
        b, hq = divmod(c, 4)
        sl = slice(hq * DPC, (hq + 1) * DPC)
        in_maps.append({
            "xT": xTb[b],
            "wq": np.ascontiguousarray(Wq[sl, :].T),
            "wk": np.ascontiguousarray(Wk[sl, :].T),
            "wv": np.ascontiguousarray(Wv[sl, :].T),
            "wo": np.ascontiguousarray(Wo[:, sl].T),
            "coss": C, "sins": Sg, "pmat": P, "mask": M, "ident": ident,
        })

    res = run_bass_kernel_spmd(nc, in_maps, list(range(8)), **_CACHE.get("runkw", {}))
    _CACHE["last_res"] = res
    out = np.zeros((B, S, D_MODEL), np.float32)
    for c in range(8):
        b = c // 4
        out[b] += res.results[c]["y"]
    return out


# revision 20
# speedup vs baseline: 1.0890x; 1.0442x over previous
"""Causal MHA (RoPE, 16 heads, D=1024, S=2048, B=2) on 8 trn2 NeuronCores.

Sharding: data-parallel over batch (2 groups of 4 cores) x tensor-parallel
over heads (4 heads / core). Each core computes q/k/v projections for its
256 output dims, RoPE, causal attention for its 4 heads, and a partial
output projection y_c = out_c @ Wo[:, slice].T. Host sums the 4 partials
per batch (row-parallel unshard).

All matmuls run in float32r (TF32-like, ~1e-4 rel err, full PE rate at
free-dim >= 256). Scores are computed transposed ([keys, q]) so the
attention @V matmul has q as its 512-wide free dim, and softmax
denominators come free as an extra ones-column in the V operand.
"""

import numpy as np

D_MODEL = 1024
S = 2048
NH = 16
HD = 64
THETA = 10000.0
HPC = 4          # heads per core
DPC = HPC * HD   # dims per core = 256
NG = 2           # dim groups of 128 (pairs of heads)
W = 512          # q-block width
NKO = D_MODEL // 128
NTC = S // 128   # 16 token chunks of 128
HALF = S // 2
MASK_VAL = -1e9

_CACHE = {}


def _build_nc():
    import concourse.bass as bass
    import concourse.tile as tile
    from concourse import bacc, mybir
    from contextlib import ExitStack

    F32 = mybir.dt.float32
    F32R = mybir.dt.float32r
    AF = mybir.ActivationFunctionType
    ts = bass.ts

    nc = bacc.Bacc(None, target_bir_lowering=False)
    xT = nc.dram_tensor("xT", [D_MODEL, S], F32, kind="ExternalInput")
    wq = nc.dram_tensor("wq", [D_MODEL, DPC], F32, kind="ExternalInput")
    wk = nc.dram_tensor("wk", [D_MODEL, DPC], F32, kind="ExternalInput")
    wv = nc.dram_tensor("wv", [D_MODEL, DPC], F32, kind="ExternalInput")
    wo = nc.dram_tensor("wo", [DPC, D_MODEL], F32, kind="ExternalInput")
    coss = nc.dram_tensor("coss", [128, S], F32, kind="ExternalInput")
    sins = nc.dram_tensor("sins", [128, S], F32, kind="ExternalInput")
    pmat = nc.dram_tensor("pmat", [128, 128], F32, kind="ExternalInput")
    mask = nc.dram_tensor("mask", [128, 128], F32, kind="ExternalInput")
    ident = nc.dram_tensor("ident", [128, 128], F32, kind="ExternalInput")
    y = nc.dram_tensor("y", [S, D_MODEL], F32, kind="ExternalOutput")

    with tile.TileContext(nc) as tc, ExitStack() as ctx:
        const = ctx.enter_context(tc.tile_pool(name="const", bufs=1))
        persist = ctx.enter_context(tc.tile_pool(name="persist", bufs=1))

        # ---- constants (DMAs deferred into phase 1 to not hog queues) --
        BF16 = mybir.dt.bfloat16
        ones_raw = const.tile([128, HPC], F32)
        nc.gpsimd.memset(ones_raw[:], 1.0)
        ones_r = const.tile([128, HPC], F32R)
        nc.vector.tensor_copy(ones_r[:], ones_raw[:])

        # persistent activations
        qT = [persist.tile([128, S], F32R, name=f"qT{g}") for g in range(NG)]
        kT = [persist.tile([128, S], F32R, name=f"kT{g}") for g in range(NG)]
        v_aug = persist.tile([128, NTC, HPC * (HD + 1)], F32R, name="v_aug")
        out_cT = [persist.tile([128, S], F32R, name=f"out_cT{g}")
                  for g in range(NG)]
        wo_r = persist.tile([128, NG, D_MODEL], F32R, name="wo_r")

        # ---- phase 1: QKV + RoPE (x streamed in 4 quarters) -----------
        with nc.named_scope("qkv"), \
             tc.tile_pool(name="qkvw", bufs=1) as wpool, \
             tc.tile_pool(name="qkv", bufs=2) as qkv_pool, \
             tc.tile_pool(name="xtr", bufs=2) as xt_pool, \
             tc.tile_pool(name="ps1v", bufs=2, space="PSUM") as ps1v, \
             tc.tile_pool(name="ps1qk", bufs=4, space="PSUM") as ps1qk, \
             tc.tile_pool(name="ps1p", bufs=2, space="PSUM") as ps1p:

            def load_round(name, dram, width):
                t = wpool.tile([128, NKO, width], F32R, name=name + "_r")
                for ko in range(NKO):
                    stg = qkv_pool.tile([128, width], F32, tag="w_stg",
                                        name="w_stg")
                    nc.sync.dma_start(stg[:], dram.ap()[ts(ko, 128), :])
                    nc.vector.tensor_copy(t[:, ko], stg[:])
                return t

            def load_x_quarter(hf):
                xr = xt_pool.tile([128, NKO, W], F32R, tag="xT_r",
                                  name="xT_r")
                for ko in range(NKO):
                    stg = qkv_pool.tile([128, W], F32, tag="x_stg",
                                        name="x_stg")
                    nc.sync.dma_start(stg[:], xT.ap()[ts(ko, 128), ts(hf, W)])
                    nc.vector.tensor_copy(xr[:, ko], stg[:])
                return xr

            wv_r = load_round("wv", wv, DPC)
            xquart = load_x_quarter(0)
            wq_r = load_round("wq", wq, DPC)
            wk_r = load_round("wk", wk, DPC)

            pm_raw = const.tile([128, 128], F32)
            nc.sync.dma_start(pm_raw[:], pmat.ap())
            pm_r = const.tile([128, 128], F32R)
            nc.scalar.copy(pm_r[:], pm_raw[:])
            msk_raw = const.tile([128, 128], F32)
            nc.sync.dma_start(msk_raw[:], mask.ap())
            msk_r = const.tile([128, 128], BF16)
            nc.scalar.copy(msk_r[:], msk_raw[:])
            id_raw = const.tile([128, 128], F32)
            nc.sync.dma_start(id_raw[:], ident.ap())
            id_r = const.tile([128, 128], BF16)
            nc.scalar.copy(id_r[:], id_raw[:])
            cs_t = const.tile([128, S], F32)
            sn_t = const.tile([128, S], F32)
            for j4 in range(4):
                nc.sync.dma_start(cs_t[:, ts(j4, W)], coss.ap()[:, ts(j4, W)])
                nc.sync.dma_start(sn_t[:, ts(j4, W)], sins.ap()[:, ts(j4, W)])
            for g2 in range(NG):
                wos = const.tile([128, D_MODEL], F32, tag="wo_stg", name="wos")
                nc.sync.dma_start(wos[:], wo.ap()[ts(g2, 128), :])
                nc.vector.tensor_copy(wo_r[:, g2], wos[:])

            for hf in range(4):
                xT_r = xquart
                if hf < 3:
                    xquart = load_x_quarter(hf + 1)

                for tl in range(W // 128):
                    tcN = hf * (W // 128) + tl
                    psv = ps1v.tile([128, DPC], F32, tag="psv", name="psv")
                    for ko in range(NKO):
                        nc.tensor.matmul(psv[:], xT_r[:, ko, ts(tl, 128)],
                                         wv_r[:, ko],
                                         start=(ko == 0), stop=(ko == NKO - 1))
                    for h in range(HPC):
                        nc.scalar.copy(
                            v_aug[:, tcN, h * (HD + 1):h * (HD + 1) + HD],
                            psv[:, ts(h, HD)])
                    nc.vector.tensor_copy(v_aug[:, tcN, HD::HD + 1], ones_r[:])

                for g in range(NG):
                    t4 = hf
                    psq = ps1qk.tile([128, W], F32, tag="psqk", name="psq")
                    for ko in range(NKO):
                        nc.tensor.matmul(
                            psq[:], wq_r[:, ko, ts(g, 128)], xT_r[:, ko],
                            start=(ko == 0), stop=(ko == NKO - 1))
                    rawq = qkv_pool.tile([128, W], F32R, tag="rawq",
                                         name="rawq")
                    nc.scalar.copy(rawq[:], psq[:])
                    psk = ps1qk.tile([128, W], F32, tag="psqk", name="psk")
                    for ko in range(NKO):
                        nc.tensor.matmul(
                            psk[:], wk_r[:, ko, ts(g, 128)], xT_r[:, ko],
                            start=(ko == 0), stop=(ko == NKO - 1))
                    rawk = qkv_pool.tile([128, W], F32R, tag="rawk",
                                         name="rawk")
                    nc.scalar.copy(rawk[:], psk[:])
                    for nm, raw, dst in (("q", rawq, qT[g]),
                                         ("k", rawk, kT[g])):
                        psp = ps1p.tile([128, W], F32, tag="psp", name="psp")
                        nc.tensor.matmul(psp[:], pm_r[:], raw[:],
                                         start=True, stop=True)
                        t1 = qkv_pool.tile([128, W], F32, tag=f"t1{nm}",
                                           name="t1")
                        nc.gpsimd.tensor_tensor(t1[:], raw[:],
                                                cs_t[:, ts(t4, W)],
                                                mybir.AluOpType.mult)
                        t2 = qkv_pool.tile([128, W], F32, tag=f"t2{nm}",
                                           name="t2")
                        nc.vector.tensor_tensor(t2[:], psp[:],
                                                sn_t[:, ts(t4, W)],
                                                mybir.AluOpType.mult)
                        nc.vector.tensor_tensor(dst[:, ts(t4, W)],
                                                t1[:], t2[:],
                                                mybir.AluOpType.add)

        # ---- phase 2: attention (+ interleaved output projection) -----
        with nc.named_scope("attn"), \
             tc.tile_pool(name="att", bufs=6) as att_pool, \
             tc.tile_pool(name="norm", bufs=3) as norm_pool, \
             tc.tile_pool(name="ps2", bufs=2, space="PSUM") as ps2, \
             tc.tile_pool(name="ps2av", bufs=1, space="PSUM") as ps2av:
            pending = []

            def emit_oproj(tcN):
                ysb = norm_pool.tile([128, D_MODEL], F32, tag="ysb",
                                     name="ysb")
                for e2 in range(2):
                    psy = ps2.tile([128, W], F32, tag="sc", name="psy")
                    for g in range(NG):
                        nc.tensor.matmul(psy[:], out_cT[g][:, ts(tcN, 128)],
                                         wo_r[:, g, ts(e2, W)],
                                         start=(g == 0), stop=(g == NG - 1),
                                         skip_group_check=True)
                    nc.vector.tensor_copy(ysb[:, ts(e2, W)], psy[:])
                nc.sync.dma_start(y.ap()[ts(tcN, 128), :], ysb[:])

            for qb in range(S // W):
                av = [ps2av.tile([HD + 1, W], F32, tag=f"av{hh}",
                                 name=f"av{hh}") for hh in range(4)]
                nkb = (qb + 1) * (W // 128)
                for kb in range(nkb):
                    if kb >= 2 and pending:
                        emit_oproj(pending.pop(0))
                    cs0 = max(0, kb * 128 - qb * W)
                    diag = kb * 128 >= qb * W
                    for g in range(NG):
                        sc = ps2.tile([128, 2 * W], F32, tag="sc", name="sc")
                        for h in range(2):
                            nc.tensor.matmul(
                                sc[:, h * W + cs0:(h + 1) * W],
                                kT[g][ts(h, HD), ts(kb, 128)],
                                qT[g][ts(h, HD), qb * W + cs0:(qb + 1) * W],
                                start=True, stop=not diag,
                                skip_group_check=True)
                            if diag:
                                nc.tensor.matmul(
                                    sc[:, h * W + cs0:h * W + cs0 + 128],
                                    id_r[:], msk_r[:],
                                    start=False, stop=True,
                                    skip_group_check=True)
                        att = att_pool.tile([128, 2 * W], F32R, tag="attw",
                                            name="att")
                        scv = sc[:].rearrange("p (h w) -> p h w", h=2)
                        atv = att[:].rearrange("p (h w) -> p h w", h=2)
                        nc.scalar.activation(atv[:, :, cs0:], scv[:, :, cs0:],
                                             AF.Exp, scale=1.0 / np.sqrt(HD))
                        for h in range(2):
                            hh = 2 * g + h
                            nc.tensor.matmul(
                                av[hh][:, cs0:],
                                v_aug[:, kb, hh * (HD + 1):
                                      (hh + 1) * (HD + 1)],
                                att[:, h * W + cs0:(h + 1) * W],
                                start=(kb == 0), stop=(kb == nkb - 1),
                                skip_group_check=True)
                rss = []
                for hh in range(4):
                    rs = norm_pool.tile([1, W], F32, tag=f"rs{hh}", name="rs")
                    nc.vector.tensor_copy(rs[:], av[hh][HD:HD + 1, :])
                    rss.append(rs)
                for hh in range(4):
                    g, h = divmod(hh, 2)
                    rec = norm_pool.tile([1, W], F32, tag="rec", name="rec")
                    nc.vector.reciprocal_approx_fast(rec[:], rss[hh][:])
                    rb = norm_pool.tile([HD, W], F32, tag="rb", name="rb")
                    nc.gpsimd.partition_broadcast(rb[:], rec[:])
                    nc.vector.tensor_tensor(
                        out_cT[g][ts(h, HD), ts(qb, W)],
                        av[hh][0:HD, :], rb[:], mybir.AluOpType.mult)
                pending.extend(qb * (W // 128) + tl for tl in range(W // 128))
            for tcN in pending:
                emit_oproj(tcN)

    nc.compile()
    return nc


def _host_inputs():
    d = HD
    inv_freq = THETA ** (-np.arange(0, d, 2, dtype=np.float64) / d)  # [32]
    t = np.arange(S, dtype=np.float64)
    ang = t[None, :] * inv_freq[:, None]          # [32, S]
    C64 = np.repeat(np.cos(ang), 2, axis=0)       # [64, S] per-dim cos
    S64 = np.repeat(np.sin(ang), 2, axis=0).copy()
    S64[0::2] *= -1.0                             # even dims: -sin
    C = np.tile(C64, (2, 1)).astype(np.float32)   # [128, S] two heads
    Sg = np.tile(S64, (2, 1)).astype(np.float32)

    P = np.zeros((128, 128), np.float32)
    idx = np.arange(128)
    P[idx ^ 1, idx] = 1.0

    M = np.where(np.arange(128)[None, :] >= np.arange(128)[:, None],
                 0.0, MASK_VAL).astype(np.float32)
    ident = np.eye(128, dtype=np.float32)
    return C, Sg, P, M, ident


def kernel(x, Wq, Wk, Wv, Wo):
    from concourse.bass_utils import run_bass_kernel_spmd

    x = np.asarray(x, np.float32)
    Wq = np.asarray(Wq, np.float32)
    Wk = np.asarray(Wk, np.float32)
    Wv = np.asarray(Wv, np.float32)
    Wo = np.asarray(Wo, np.float32)
    B = x.shape[0]

    if "nc" not in _CACHE:
        _CACHE["nc"] = _build_nc()
    nc = _CACHE["nc"]

    C, Sg, P, M, ident = _host_inputs()
    xTb = [np.ascontiguousarray(x[b].T) for b in range(B)]
    in_maps = []
    for c in range(8):
        b, hq = divmod(c, 4)
        sl = slice(hq * DPC, (hq + 1) * DPC)
        in_maps.append({
            "xT": xTb[b],
            "wq": np.ascontiguousarray(Wq[sl, :].T),
            "wk": np.ascontiguousarray(Wk[sl, :].T),
            "wv": np.ascontiguousarray(Wv[sl, :].T),
            "wo": np.ascontiguousarray(Wo[:, sl].T),
            "coss": C, "sins": Sg, "pmat": P, "mask": M, "ident": ident,
        })

    res = run_bass_kernel_spmd(nc, in_maps, list(range(8)), **_CACHE.get("runkw", {}))
    _CACHE["last_res"] = res
    out = np.zeros((B, S, D_MODEL), np.float32)
    for c in range(8):
        b = c // 4
        out[b] += res.results[c]["y"]
    return out


# revision 22
# speedup vs baseline: 1.1314x; 1.0389x over previous
"""Causal MHA (RoPE, 16 heads, D=1024, S=2048, B=2) on 8 trn2 NeuronCores.

Sharding: data-parallel over batch (2 groups of 4 cores) x tensor-parallel
over heads (4 heads / core). Each core computes q/k/v projections for its
256 output dims, RoPE, causal attention for its 4 heads, and a partial
output projection y_c = out_c @ Wo[:, slice].T. Host sums the 4 partials
per batch (row-parallel unshard).

All matmuls run in float32r (TF32-like, ~1e-4 rel err, full PE rate at
free-dim >= 256). Scores are computed transposed ([keys, q]) so the
attention @V matmul has q as its 512-wide free dim, and softmax
denominators come free as an extra ones-column in the V operand.
"""

import numpy as np

D_MODEL = 1024
S = 2048
NH = 16
HD = 64
THETA = 10000.0
HPC = 4          # heads per core
DPC = HPC * HD   # dims per core = 256
NG = 2           # dim groups of 128 (pairs of heads)
W = 512          # q-block width
NKO = D_MODEL // 128
NTC = S // 128   # 16 token chunks of 128
HALF = S // 2
MASK_VAL = -1e9

_CACHE = {}


def _build_nc():
    import concourse.bass as bass
    import concourse.tile as tile
    from concourse import bacc, mybir
    from contextlib import ExitStack

    F32 = mybir.dt.float32
    F32R = mybir.dt.float32r
    AF = mybir.ActivationFunctionType
    ts = bass.ts

    nc = bacc.Bacc(None, target_bir_lowering=False)
    xT = nc.dram_tensor("xT", [D_MODEL, S], F32, kind="ExternalInput")
    wq = nc.dram_tensor("wq", [D_MODEL, DPC], F32, kind="ExternalInput")
    wk = nc.dram_tensor("wk", [D_MODEL, DPC], F32, kind="ExternalInput")
    wv = nc.dram_tensor("wv", [D_MODEL, DPC], F32, kind="ExternalInput")
    wo = nc.dram_tensor("wo", [DPC, D_MODEL], F32, kind="ExternalInput")
    coss = nc.dram_tensor("coss", [128, S], F32, kind="ExternalInput")
    sins = nc.dram_tensor("sins", [128, S], F32, kind="ExternalInput")
    pmat = nc.dram_tensor("pmat", [128, 128], F32, kind="ExternalInput")
    mask = nc.dram_tensor("mask", [128, 128], F32, kind="ExternalInput")
    ident = nc.dram_tensor("ident", [128, 128], F32, kind="ExternalInput")
    y = nc.dram_tensor("y", [S, D_MODEL], F32, kind="ExternalOutput")

    with tile.TileContext(nc) as tc, ExitStack() as ctx:
        const = ctx.enter_context(tc.tile_pool(name="const", bufs=1))
        persist = ctx.enter_context(tc.tile_pool(name="persist", bufs=1))

        # ---- constants (DMAs deferred into phase 1 to not hog queues) --
        BF16 = mybir.dt.bfloat16
        ones_raw = const.tile([128, HPC], F32)
        nc.gpsimd.memset(ones_raw[:], 1.0)
        ones_r = const.tile([128, HPC], F32R)
        nc.vector.tensor_copy(ones_r[:], ones_raw[:])

        # persistent activations
        qT = [persist.tile([128, S], F32R, name=f"qT{g}") for g in range(NG)]
        kT = [persist.tile([128, S], F32R, name=f"kT{g}") for g in range(NG)]
        v_aug = persist.tile([128, NTC, HPC * (HD + 1)], F32R, name="v_aug")
        out_cT = [persist.tile([128, S], F32R, name=f"out_cT{g}")
                  for g in range(NG)]
        wo_r = persist.tile([128, NG, D_MODEL], F32R, name="wo_r")

        # ---- phase 1: QKV + RoPE (x streamed in 4 quarters) -----------
        with nc.named_scope("qkv"), \
             tc.tile_pool(name="qkvw", bufs=1) as wpool, \
             tc.tile_pool(name="qkv", bufs=2) as qkv_pool, \
             tc.tile_pool(name="xtr", bufs=2) as xt_pool, \
             tc.tile_pool(name="ps1v", bufs=2, space="PSUM") as ps1v, \
             tc.tile_pool(name="ps1qk", bufs=4, space="PSUM") as ps1qk, \
             tc.tile_pool(name="ps1p", bufs=2, space="PSUM") as ps1p:

            def load_round(name, dram, width):
                t = wpool.tile([128, NKO, width], F32R, name=name + "_r")
                for ko in range(NKO):
                    stg = qkv_pool.tile([128, width], F32, tag="w_stg",
                                        name="w_stg")
                    nc.sync.dma_start(stg[:], dram.ap()[ts(ko, 128), :])
                    nc.vector.tensor_copy(t[:, ko], stg[:])
                return t

            def load_x_quarter(hf):
                xr = xt_pool.tile([128, NKO, W], F32R, tag="xT_r",
                                  name="xT_r")
                for ko in range(NKO):
                    stg = qkv_pool.tile([128, W], F32, tag="x_stg",
                                        name="x_stg")
                    nc.sync.dma_start(stg[:], xT.ap()[ts(ko, 128), ts(hf, W)])
                    nc.vector.tensor_copy(xr[:, ko], stg[:])
                return xr

            wv_r = load_round("wv", wv, DPC)
            xquart = load_x_quarter(0)
            wq_r = load_round("wq", wq, DPC)
            wk_r = load_round("wk", wk, DPC)

            pm_raw = const.tile([128, 128], F32)
            nc.sync.dma_start(pm_raw[:], pmat.ap())
            pm_r = const.tile([128, 128], F32R)
            nc.scalar.copy(pm_r[:], pm_raw[:])
            msk_raw = const.tile([128, 128], F32)
            nc.sync.dma_start(msk_raw[:], mask.ap())
            msk_r = const.tile([128, 128], BF16)
            nc.scalar.copy(msk_r[:], msk_raw[:])
            id_raw = const.tile([128, 128], F32)
            nc.sync.dma_start(id_raw[:], ident.ap())
            id_r = const.tile([128, 128], BF16)
            nc.scalar.copy(id_r[:], id_raw[:])
            cs_t = const.tile([128, S], F32)
            sn_t = const.tile([128, S], F32)
            for j4 in range(4):
                nc.sync.dma_start(cs_t[:, ts(j4, W)], coss.ap()[:, ts(j4, W)])
                nc.sync.dma_start(sn_t[:, ts(j4, W)], sins.ap()[:, ts(j4, W)])
            for g2 in range(NG):
                wos = const.tile([128, D_MODEL], F32, tag="wo_stg", name="wos")
                nc.sync.dma_start(wos[:], wo.ap()[ts(g2, 128), :])
                nc.vector.tensor_copy(wo_r[:, g2], wos[:])

            for hf in range(4):
                xT_r = xquart
                if hf < 3:
                    xquart = load_x_quarter(hf + 1)

                for tl in range(W // 128):
                    tcN = hf * (W // 128) + tl
                    psv = ps1v.tile([128, DPC], F32, tag="psv", name="psv")
                    for ko in range(NKO):
                        nc.tensor.matmul(psv[:], xT_r[:, ko, ts(tl, 128)],
                                         wv_r[:, ko],
                                         start=(ko == 0), stop=(ko == NKO - 1))
                    nc.scalar.copy(
                        v_aug[:, tcN].rearrange("p (h c) -> p h c",
                                                h=HPC)[:, :, 0:HD],
                        psv[:].rearrange("p (h c) -> p h c", h=HPC))
                    nc.vector.tensor_copy(v_aug[:, tcN, HD::HD + 1], ones_r[:])

                for g in range(NG):
                    t4 = hf
                    psq = ps1qk.tile([128, W], F32, tag="psqk", name="psq")
                    for ko in range(NKO):
                        nc.tensor.matmul(
                            psq[:], wq_r[:, ko, ts(g, 128)], xT_r[:, ko],
                            start=(ko == 0), stop=(ko == NKO - 1))
                    rawq = qkv_pool.tile([128, W], F32R, tag="rawq",
                                         name="rawq")
                    nc.scalar.copy(rawq[:], psq[:])
                    psk = ps1qk.tile([128, W], F32, tag="psqk", name="psk")
                    for ko in range(NKO):
                        nc.tensor.matmul(
                            psk[:], wk_r[:, ko, ts(g, 128)], xT_r[:, ko],
                            start=(ko == 0), stop=(ko == NKO - 1))
                    rawk = qkv_pool.tile([128, W], F32R, tag="rawk",
                                         name="rawk")
                    nc.scalar.copy(rawk[:], psk[:])
                    for nm, raw, dst in (("q", rawq, qT[g]),
                                         ("k", rawk, kT[g])):
                        psp = ps1p.tile([128, W], F32, tag="psp", name="psp")
                        nc.tensor.matmul(psp[:], pm_r[:], raw[:],
                                         start=True, stop=True)
                        t1 = qkv_pool.tile([128, W], F32, tag=f"t1{nm}",
                                           name="t1")
                        nc.gpsimd.tensor_tensor(t1[:], raw[:],
                                                cs_t[:, ts(t4, W)],
                                                mybir.AluOpType.mult)
                        t2 = qkv_pool.tile([128, W], F32, tag=f"t2{nm}",
                                           name="t2")
                        nc.vector.tensor_tensor(t2[:], psp[:],
                                                sn_t[:, ts(t4, W)],
                                                mybir.AluOpType.mult)
                        nc.vector.tensor_tensor(dst[:, ts(t4, W)],
                                                t1[:], t2[:],
                                                mybir.AluOpType.add)

        # ---- phase 2: attention (+ interleaved output projection) -----
        with nc.named_scope("attn"), \
             tc.tile_pool(name="att", bufs=6) as att_pool, \
             tc.tile_pool(name="norm", bufs=3) as norm_pool, \
             tc.tile_pool(name="ps2", bufs=2, space="PSUM") as ps2, \
             tc.tile_pool(name="ps2av", bufs=1, space="PSUM") as ps2av:
            pending = []

            def emit_oproj(tcN):
                ysb = norm_pool.tile([128, D_MODEL], F32, tag="ysb",
                                     name="ysb")
                for e2 in range(2):
                    psy = ps2.tile([128, W], F32, tag="sc", name="psy")
                    for g in range(NG):
                        nc.tensor.matmul(psy[:], out_cT[g][:, ts(tcN, 128)],
                                         wo_r[:, g, ts(e2, W)],
                                         start=(g == 0), stop=(g == NG - 1),
                                         skip_group_check=True)
                    nc.vector.tensor_copy(ysb[:, ts(e2, W)], psy[:])
                nc.sync.dma_start(y.ap()[ts(tcN, 128), :], ysb[:])

            LAG = 2
            for qb in range(S // W):
                av = [ps2av.tile([HD + 1, W], F32, tag=f"av{hh}",
                                 name=f"av{hh}") for hh in range(4)]
                nkb = (qb + 1) * (W // 128)
                attq = []

                def emit_av(entry, nkb=nkb, av=av):
                    kb, cs0, atts = entry
                    for g in range(NG):
                        for h in range(2):
                            hh = 2 * g + h
                            nc.tensor.matmul(
                                av[hh][:, cs0:],
                                v_aug[:, kb, hh * (HD + 1):
                                      (hh + 1) * (HD + 1)],
                                atts[g][:, h * W + cs0:(h + 1) * W],
                                start=(kb == 0), stop=(kb == nkb - 1),
                                skip_group_check=True)

                for kb in range(nkb):
                    if kb >= 2 and pending:
                        emit_oproj(pending.pop(0))
                    cs0 = max(0, kb * 128 - qb * W)
                    diag = kb * 128 >= qb * W
                    atts = []
                    for g in range(NG):
                        sc = ps2.tile([128, 2 * W], F32, tag="sc", name="sc")
                        for h in range(2):
                            nc.tensor.matmul(
                                sc[:, h * W + cs0:(h + 1) * W],
                                kT[g][ts(h, HD), ts(kb, 128)],
                                qT[g][ts(h, HD), qb * W + cs0:(qb + 1) * W],
                                start=True, stop=not diag,
                                skip_group_check=True)
                            if diag:
                                nc.tensor.matmul(
                                    sc[:, h * W + cs0:h * W + cs0 + 128],
                                    id_r[:], msk_r[:],
                                    start=False, stop=True,
                                    skip_group_check=True)
                        att = att_pool.tile([128, 2 * W], F32R, tag="attw",
                                            name="att")
                        scv = sc[:].rearrange("p (h w) -> p h w", h=2)
                        atv = att[:].rearrange("p (h w) -> p h w", h=2)
                        nc.scalar.activation(atv[:, :, cs0:], scv[:, :, cs0:],
                                             AF.Exp, scale=1.0 / np.sqrt(HD))
                        atts.append(att)
                    attq.append((kb, cs0, atts))
                    if len(attq) > LAG:
                        emit_av(attq.pop(0))
                while attq:
                    emit_av(attq.pop(0))
                rss = []
                for hh in range(4):
                    rs = norm_pool.tile([1, W], F32, tag=f"rs{hh}", name="rs")
                    nc.vector.tensor_copy(rs[:], av[hh][HD:HD + 1, :])
                    rss.append(rs)
                for hh in range(4):
                    g, h = divmod(hh, 2)
                    rec = norm_pool.tile([1, W], F32, tag="rec", name="rec")
                    nc.vector.reciprocal_approx_fast(rec[:], rss[hh][:])
                    rb = norm_pool.tile([HD, W], F32, tag="rb", name="rb")
                    nc.gpsimd.partition_broadcast(rb[:], rec[:])
                    nc.vector.tensor_tensor(
                        out_cT[g][ts(h, HD), ts(qb, W)],
                        av[hh][0:HD, :], rb[:], mybir.AluOpType.mult)
                pending.extend(qb * (W // 128) + tl for tl in range(W // 128))
            for tcN in pending:
                emit_oproj(tcN)

    nc.compile()
    return nc


def _host_inputs():
    d = HD
    inv_freq = THETA ** (-np.arange(0, d, 2, dtype=np.float64) / d)  # [32]
    t = np.arange(S, dtype=np.float64)
    ang = t[None, :] * inv_freq[:, None]          # [32, S]
    C64 = np.repeat(np.cos(ang), 2, axis=0)       # [64, S] per-dim cos
    S64 = np.repeat(np.sin(ang), 2, axis=0).copy()
    S64[0::2] *= -1.0                             # even dims: -sin
    C = np.tile(C64, (2, 1)).astype(np.float32)   # [128, S] two heads
    Sg = np.tile(S64, (2, 1)).astype(np.float32)

    P = np.zeros((128, 128), np.float32)
    idx = np.arange(128)
    P[idx ^ 1, idx] = 1.0

    M = np.where(np.arange(128)[None, :] >= np.arange(128)[:, None],
                 0.0, MASK_VAL).astype(np.float32)
    ident = np.eye(128, dtype=np.float32)
    return C, Sg, P, M, ident


def kernel(x, Wq, Wk, Wv, Wo):
    from concourse.bass_utils import run_bass_kernel_spmd

    x = np.asarray(x, np.float32)
    Wq = np.asarray(Wq, np.float32)
    Wk = np.asarray(Wk, np.float32)
    Wv = np.asarray(Wv, np.float32)
    Wo = np.asarray(Wo, np.float32)
    B = x.shape[0]

    if "nc" not in _CACHE:
        _CACHE["nc"] = _build_nc()
    nc = _CACHE["nc"]

    C, Sg, P, M, ident = _host_inputs()
    xTb = [np.ascontiguousarray(x[b].T) for b in range(B)]
    in_maps = []
    for c in range(8):
        b, hq = divmod(c, 4)
        sl = slice(hq * DPC, (hq + 1) * DPC)
        in_maps.append({
            "xT": xTb[b],
            "wq": np.ascontiguousarray(Wq[sl, :].T),
            "wk": np.ascontiguousarray(Wk[sl, :].T),
            "wv": np.ascontiguousarray(Wv[sl, :].T),
            "wo": np.ascontiguousarray(Wo[:, sl].T),
            "coss": C, "sins": Sg, "pmat": P, "mask": M, "ident": ident,
        })

    res = run_bass_kernel_spmd(nc, in_maps, list(range(8)), **_CACHE.get("runkw", {}))
    _CACHE["last_res"] = res
    out = np.zeros((B, S, D_MODEL), np.float32)
    for c in range(8):
        b = c // 4
        out[b] += res.results[c]["y"]
    return out


# revision 23
# speedup vs baseline: 1.1978x; 1.0587x over previous
"""Causal MHA (RoPE, 16 heads, D=1024, S=2048, B=2) on 8 trn2 NeuronCores.

Sharding: data-parallel over batch (2 groups of 4 cores) x tensor-parallel
over heads (4 heads / core). Each core computes q/k/v projections for its
256 output dims, RoPE, causal attention for its 4 heads, and a partial
output projection y_c = out_c @ Wo[:, slice].T. Host sums the 4 partials
per batch (row-parallel unshard).

All matmuls run in float32r (TF32-like, ~1e-4 rel err, full PE rate at
free-dim >= 256). Scores are computed transposed ([keys, q]) so the
attention @V matmul has q as its 512-wide free dim, and softmax
denominators come free as an extra ones-column in the V operand.
"""

import numpy as np

D_MODEL = 1024
S = 2048
NH = 16
HD = 64
THETA = 10000.0
HPC = 4          # heads per core
DPC = HPC * HD   # dims per core = 256
NG = 2           # dim groups of 128 (pairs of heads)
W = 512          # q-block width
NKO = D_MODEL // 128
NTC = S // 128   # 16 token chunks of 128
HALF = S // 2
MASK_VAL = -1e9

_CACHE = {}


def _build_nc():
    import concourse.bass as bass
    import concourse.tile as tile
    from concourse import bacc, mybir
    from contextlib import ExitStack

    F32 = mybir.dt.float32
    F32R = mybir.dt.float32r
    AF = mybir.ActivationFunctionType
    ts = bass.ts

    nc = bacc.Bacc(None, target_bir_lowering=False)
    xT = nc.dram_tensor("xT", [D_MODEL, S], F32, kind="ExternalInput")
    wq = nc.dram_tensor("wq", [D_MODEL, DPC], F32, kind="ExternalInput")
    wk = nc.dram_tensor("wk", [D_MODEL, DPC], F32, kind="ExternalInput")
    wv = nc.dram_tensor("wv", [D_MODEL, DPC], F32, kind="ExternalInput")
    wo = nc.dram_tensor("wo", [DPC, D_MODEL], F32, kind="ExternalInput")
    coss = nc.dram_tensor("coss", [128, S], F32, kind="ExternalInput")
    sins = nc.dram_tensor("sins", [128, S], F32, kind="ExternalInput")
    pmat = nc.dram_tensor("pmat", [128, 128], F32, kind="ExternalInput")
    mask = nc.dram_tensor("mask", [128, 128], F32, kind="ExternalInput")
    ident = nc.dram_tensor("ident", [128, 128], F32, kind="ExternalInput")
    y = nc.dram_tensor("y", [S, D_MODEL], F32, kind="ExternalOutput")

    with tile.TileContext(nc) as tc, ExitStack() as ctx:
        const = ctx.enter_context(tc.tile_pool(name="const", bufs=1))
        persist = ctx.enter_context(tc.tile_pool(name="persist", bufs=1))

        # ---- constants (DMAs deferred into phase 1 to not hog queues) --
        BF16 = mybir.dt.bfloat16
        ones_raw = const.tile([128, HPC], F32)
        nc.gpsimd.memset(ones_raw[:], 1.0)
        ones_r = const.tile([128, HPC], F32R)
        nc.vector.tensor_copy(ones_r[:], ones_raw[:])

        # persistent activations
        qT = [persist.tile([128, S], F32R, name=f"qT{g}") for g in range(NG)]
        kT = [persist.tile([128, S], F32R, name=f"kT{g}") for g in range(NG)]
        v_aug = persist.tile([128, NTC, HPC * (HD + 1)], F32R, name="v_aug")
        out_cT = [persist.tile([128, S], F32R, name=f"out_cT{g}")
                  for g in range(NG)]
        wo_r = persist.tile([128, NG, D_MODEL], F32R, name="wo_r")

        # ---- phase 1: QKV + RoPE (x streamed in 4 quarters) -----------
        with nc.named_scope("qkv"), \
             tc.tile_pool(name="qkvw", bufs=1) as wpool, \
             tc.tile_pool(name="qkv", bufs=3) as qkv_pool, \
             tc.tile_pool(name="xtr", bufs=2) as xt_pool, \
             tc.tile_pool(name="ps1v", bufs=2, space="PSUM") as ps1v, \
             tc.tile_pool(name="ps1qk", bufs=4, space="PSUM") as ps1qk, \
             tc.tile_pool(name="ps1p", bufs=2, space="PSUM") as ps1p:

            def load_round(name, dram, width):
                t = wpool.tile([128, NKO, width], F32R, name=name + "_r")
                for ko in range(NKO):
                    stg = qkv_pool.tile([128, width], F32, tag="w_stg",
                                        name="w_stg")
                    nc.sync.dma_start(stg[:], dram.ap()[ts(ko, 128), :])
                    nc.vector.tensor_copy(t[:, ko], stg[:])
                return t

            def load_x_quarter(hf):
                xr = xt_pool.tile([128, NKO, W], F32R, tag="xT_r",
                                  name="xT_r")
                for ko in range(NKO):
                    stg = qkv_pool.tile([128, W], F32, tag="x_stg",
                                        name="x_stg")
                    nc.sync.dma_start(stg[:], xT.ap()[ts(ko, 128), ts(hf, W)])
                    nc.vector.tensor_copy(xr[:, ko], stg[:])
                return xr

            wv_r = load_round("wv", wv, DPC)
            xquart = load_x_quarter(0)
            wq_r = load_round("wq", wq, DPC)
            wk_r = load_round("wk", wk, DPC)

            pm_raw = const.tile([128, 128], F32)
            nc.sync.dma_start(pm_raw[:], pmat.ap())
            pm_r = const.tile([128, 128], F32R)
            nc.scalar.copy(pm_r[:], pm_raw[:])
            msk_raw = const.tile([128, 128], F32)
            nc.sync.dma_start(msk_raw[:], mask.ap())
            msk_r = const.tile([128, 128], BF16)
            nc.scalar.copy(msk_r[:], msk_raw[:])
            id_raw = const.tile([128, 128], F32)
            nc.sync.dma_start(id_raw[:], ident.ap())
            id_r = const.tile([128, 128], BF16)
            nc.scalar.copy(id_r[:], id_raw[:])
            cs_t = const.tile([128, S], F32)
            sn_t = const.tile([128, S], F32)
            for j4 in range(4):
                nc.sync.dma_start(cs_t[:, ts(j4, W)], coss.ap()[:, ts(j4, W)])
                nc.sync.dma_start(sn_t[:, ts(j4, W)], sins.ap()[:, ts(j4, W)])
            for g2 in range(NG):
                wos = const.tile([128, D_MODEL], F32, tag="wo_stg", name="wos")
                nc.sync.dma_start(wos[:], wo.ap()[ts(g2, 128), :])
                nc.vector.tensor_copy(wo_r[:, g2], wos[:])

            for hf in range(4):
                xT_r = xquart
                if hf < 3:
                    xquart = load_x_quarter(hf + 1)

                for tl in range(W // 128):
                    tcN = hf * (W // 128) + tl
                    psv = ps1v.tile([128, DPC], F32, tag="psv", name="psv")
                    for ko in range(NKO):
                        nc.tensor.matmul(psv[:], xT_r[:, ko, ts(tl, 128)],
                                         wv_r[:, ko],
                                         start=(ko == 0), stop=(ko == NKO - 1))
                    nc.scalar.copy(
                        v_aug[:, tcN].rearrange("p (h c) -> p h c",
                                                h=HPC)[:, :, 0:HD],
                        psv[:].rearrange("p (h c) -> p h c", h=HPC))
                    nc.vector.tensor_copy(v_aug[:, tcN, HD::HD + 1], ones_r[:])

                for g in range(NG):
                    t4 = hf
                    psq = ps1qk.tile([128, W], F32, tag="psqk", name="psq")
                    for ko in range(NKO):
                        nc.tensor.matmul(
                            psq[:], wq_r[:, ko, ts(g, 128)], xT_r[:, ko],
                            start=(ko == 0), stop=(ko == NKO - 1))
                    rawq = qkv_pool.tile([128, W], F32R, tag="rawq",
                                         name="rawq")
                    nc.scalar.copy(rawq[:], psq[:])
                    psk = ps1qk.tile([128, W], F32, tag="psqk", name="psk")
                    for ko in range(NKO):
                        nc.tensor.matmul(
                            psk[:], wk_r[:, ko, ts(g, 128)], xT_r[:, ko],
                            start=(ko == 0), stop=(ko == NKO - 1))
                    rawk = qkv_pool.tile([128, W], F32R, tag="rawk",
                                         name="rawk")
                    nc.scalar.copy(rawk[:], psk[:])
                    for nm, raw, dst in (("q", rawq, qT[g]),
                                         ("k", rawk, kT[g])):
                        psp = ps1p.tile([128, W], F32, tag="psp", name="psp")
                        nc.tensor.matmul(psp[:], pm_r[:], raw[:],
                                         start=True, stop=True)
                        t1 = qkv_pool.tile([128, W], F32, tag=f"t1{nm}",
                                           name="t1")
                        nc.gpsimd.tensor_tensor(t1[:], raw[:],
                                                cs_t[:, ts(t4, W)],
                                                mybir.AluOpType.mult)
                        t2 = qkv_pool.tile([128, W], F32, tag=f"t2{nm}",
                                           name="t2")
                        nc.vector.tensor_tensor(t2[:], psp[:],
                                                sn_t[:, ts(t4, W)],
                                                mybir.AluOpType.mult)
                        nc.vector.tensor_tensor(dst[:, ts(t4, W)],
                                                t1[:], t2[:],
                                                mybir.AluOpType.add)

        # ---- phase 2: attention (+ interleaved output projection) -----
        with nc.named_scope("attn"), \
             tc.tile_pool(name="att", bufs=6) as att_pool, \
             tc.tile_pool(name="norm", bufs=3) as norm_pool, \
             tc.tile_pool(name="ps2", bufs=2, space="PSUM") as ps2, \
             tc.tile_pool(name="ps2av", bufs=1, space="PSUM") as ps2av:
            pending = []

            def emit_oproj(tcN):
                ysb = norm_pool.tile([128, D_MODEL], F32, tag="ysb",
                                     name="ysb")
                for e2 in range(2):
                    psy = ps2.tile([128, W], F32, tag="sc", name="psy")
                    for g in range(NG):
                        nc.tensor.matmul(psy[:], out_cT[g][:, ts(tcN, 128)],
                                         wo_r[:, g, ts(e2, W)],
                                         start=(g == 0), stop=(g == NG - 1),
                                         skip_group_check=True)
                    nc.vector.tensor_copy(ysb[:, ts(e2, W)], psy[:])
                nc.sync.dma_start(y.ap()[ts(tcN, 128), :], ysb[:])

            LAG = 2
            for qb in range(S // W):
                av = [ps2av.tile([HD + 1, W], F32, tag=f"av{hh}",
                                 name=f"av{hh}") for hh in range(4)]
                nkb = (qb + 1) * (W // 128)
                attq = []

                def emit_av(entry, nkb=nkb, av=av):
                    kb, cs0, atts = entry
                    for g in range(NG):
                        for h in range(2):
                            hh = 2 * g + h
                            nc.tensor.matmul(
                                av[hh][:, cs0:],
                                v_aug[:, kb, hh * (HD + 1):
                                      (hh + 1) * (HD + 1)],
                                atts[g][:, h * W + cs0:(h + 1) * W],
                                start=(kb == 0), stop=(kb == nkb - 1),
                                skip_group_check=True)

                for kb in range(nkb):
                    if kb >= 2 and pending:
                        emit_oproj(pending.pop(0))
                    cs0 = max(0, kb * 128 - qb * W)
                    diag = kb * 128 >= qb * W
                    atts = []
                    for g in range(NG):
                        sc = ps2.tile([128, 2 * W], F32, tag="sc", name="sc")
                        for h in range(2):
                            nc.tensor.matmul(
                                sc[:, h * W + cs0:(h + 1) * W],
                                kT[g][ts(h, HD), ts(kb, 128)],
                                qT[g][ts(h, HD), qb * W + cs0:(qb + 1) * W],
                                start=True, stop=not diag,
                                skip_group_check=True)
                            if diag:
                                nc.tensor.matmul(
                                    sc[:, h * W + cs0:h * W + cs0 + 128],
                                    id_r[:], msk_r[:],
                                    start=False, stop=True,
                                    skip_group_check=True)
                        att = att_pool.tile([128, 2 * W], F32R, tag="attw",
                                            name="att")
                        scv = sc[:].rearrange("p (h w) -> p h w", h=2)
                        atv = att[:].rearrange("p (h w) -> p h w", h=2)
                        nc.scalar.activation(atv[:, :, cs0:], scv[:, :, cs0:],
                                             AF.Exp, scale=1.0 / np.sqrt(HD))
                        atts.append(att)
                    attq.append((kb, cs0, atts))
                    if len(attq) > LAG:
                        emit_av(attq.pop(0))
                while attq:
                    emit_av(attq.pop(0))
                rss = []
                for hh in range(4):
                    rs = norm_pool.tile([1, W], F32, tag=f"rs{hh}", name="rs")
                    nc.vector.tensor_copy(rs[:], av[hh][HD:HD + 1, :])
                    rss.append(rs)
                for hh in range(4):
                    g, h = divmod(hh, 2)
                    rec = norm_pool.tile([1, W], F32, tag="rec", name="rec")
                    nc.vector.reciprocal_approx_fast(rec[:], rss[hh][:])
                    rb = norm_pool.tile([HD, W], F32, tag="rb", name="rb")
                    nc.gpsimd.partition_broadcast(rb[:], rec[:])
                    nc.vector.tensor_tensor(
                        out_cT[g][ts(h, HD), ts(qb, W)],
                        av[hh][0:HD, :], rb[:], mybir.AluOpType.mult)
                pending.extend(qb * (W // 128) + tl for tl in range(W // 128))
            for tcN in pending:
                emit_oproj(tcN)

    nc.compile()
    return nc


def _host_inputs():
    d = HD
    inv_freq = THETA ** (-np.arange(0, d, 2, dtype=np.float64) / d)  # [32]
    t = np.arange(S, dtype=np.float64)
    ang = t[None, :] * inv_freq[:, None]          # [32, S]
    C64 = np.repeat(np.cos(ang), 2, axis=0)       # [64, S] per-dim cos
    S64 = np.repeat(np.sin(ang), 2, axis=0).copy()
    S64[0::2] *= -1.0                             # even dims: -sin
    C = np.tile(C64, (2, 1)).astype(np.float32)   # [128, S] two heads
    Sg = np.tile(S64, (2, 1)).astype(np.float32)

    P = np.zeros((128, 128), np.float32)
    idx = np.arange(128)
    P[idx ^ 1, idx] = 1.0

    M = np.where(np.arange(128)[None, :] >= np.arange(128)[:, None],
                 0.0, MASK_VAL).astype(np.float32)
    ident = np.eye(128, dtype=np.float32)
    return C, Sg, P, M, ident


def kernel(x, Wq, Wk, Wv, Wo):
    from concourse.bass_utils import run_bass_kernel_spmd

    x = np.asarray(x, np.float32)
    Wq = np.asarray(Wq, np.float32)
    Wk = np.asarray(Wk, np.float32)
    Wv = np.asarray(Wv, np.float32)
    Wo = np.asarray(Wo, np.float32)
    B = x.shape[0]

    if "nc" not in _CACHE:
        _CACHE["nc"] = _build_nc()
    nc = _CACHE["nc"]

    C, Sg, P, M, ident = _host_inputs()
    xTb = [np.ascontiguousarray(x[b].T) for b in range(B)]
    in_maps = []
    for c in range(8):
        b, hq = divmod(c, 4)
        sl = slice(hq * DPC, (hq + 1) * DPC)
        in_maps.append({
            "xT": xTb[b],
            "wq": np.ascontiguousarray(Wq[sl, :].T),
            "wk": np.ascontiguousarray(Wk[sl, :].T),
            "wv": np.ascontiguousarray(Wv[sl, :].T),
            "wo": np.ascontiguousarray(Wo[:, sl].T),
            "coss": C, "sins": Sg, "pmat": P, "mask": M, "ident": ident,
        })

    res = run_bass_kernel_spmd(nc, in_maps, list(range(8)), **_CACHE.get("runkw", {}))
    _CACHE["last_res"] = res
    out = np.zeros((B, S, D_MODEL), np.float32)
    for c in range(8):
        b = c // 4
        out[b] += res.results[c]["y"]
    return out
